# revision 6
# baseline (speedup 1.0000x reference)
"""CVA module (deformable cross-attention) Trainium2 Bass kernel.

Full inputs in, full outputs out. Data-parallel over batch: B=8 frames
-> 8 NeuronCores, one frame per core.

Per-core pipeline:
  q = Wq@x1+bq                                           PE (fp32r)
  offset net: depthwise5x5(q_g) via 25 diagonal-matmul taps -> PSUM,
  LayerNorm over channels via ones-matmul stats + selector broadcasts,
  gelu(tanh)                                             PE + DVE + ACT
  positions -> corner indices (rint floor) + lerp weights     DVE
  gather x2 columns (ap_gather on GPSIMD), lerp combine       GPSIMD+PE+DVE
  k = Wk@samp+bk; vT = samp^T@Wv^T+bv (ones-augmented cols)   PE
  scoresT[k,q] = kf^T qf  (4-head row-tiled 32x128 tiles)     PE
  p = exp(scale*scoresT)  (batched [128,2048] ops)            ACT
  attnT[q,(h,d|s)] = pT^T @ vTa (q-tiled; ones col -> sums)   PE
  normalize via per-partition reciprocal + free-dim broadcast DVE
  PE-transpose attnT -> attn[d,q]; y = Wo@attn+bo; y += x1    PE + DVE
"""
import sys

sys.path.insert(0, '/opt/trn_rl_repo')

from contextlib import ExitStack

import numpy as np

import concourse.bass as bass
import concourse.mybir as mybir
import concourse.tile as tile
from concourse import bacc, bass_utils, library_config

dt = mybir.dt
f32r = dt.float32r
Alu = mybir.AluOpType
Act = mybir.ActivationFunctionType

B, C, H, W = 8, 384, 32, 32
HEADS, GROUPS = 12, 3
CG = C // GROUPS      # 128
HD = C // HEADS       # 32
NS = H * W            # 1024
EPS = 1e-5
SCALE = 1.0 / float(np.sqrt(HD))
NCORES = 8
QQ = 4                # q quarters of 256
KC = 8                # k chunks of 128

_CACHED = {}


def _host_consts():
    if 'consts' in _CACHED:
        return _CACHED['consts']
    xs = (np.arange(W, dtype=np.float32) + 0.5) / W * 2.0 - 1.0
    ys = (np.arange(H, dtype=np.float32) + 0.5) / H * 2.0 - 1.0
    gy, gx = np.meshgrid(ys, xs, indexing='ij')
    grid2x = np.broadcast_to((gx.reshape(-1) * 15.5 + 15.5)[None, :], (3, NS)).copy()
    grid2y = np.broadcast_to((gy.reshape(-1) * 15.5 + 15.5)[None, :], (3, NS)).copy()
    ones_row = np.ones((1, NS), np.float32)
    ones_col12 = np.zeros((128, 2 * HEADS), np.float32)
    ones_col12[:, 0::2] = 1.0
    zeros_pad = np.zeros((1, 32 * 36), np.float32)
    ident = np.eye(128, dtype=np.float32)
    e3div = np.zeros((128, 9), np.float32)
    for g in range(3):
        e3div[:, 3 * g + g] = 1.0 / 128.0
    sel3 = np.zeros((3, 3 * 128), np.float32)
    for g in range(3):
        sel3[g, 128 * g:128 * (g + 1)] = 1.0
    _CACHED['consts'] = (grid2x, grid2y, ones_row, ones_col12, ident, e3div, sel3, zeros_pad)
    return _CACHED['consts']


def _host_weights(inp):
    Wq, Wk, Wv, Wo = [np.asarray(inp[k], np.float32) for k in ('Wq', 'Wk', 'Wv', 'Wo')]
    off_dw = np.asarray(inp['off_dw'], np.float32)
    off_w = np.asarray(inp['off_w'], np.float32)
    ln_g = np.asarray(inp['ln_g'], np.float32)
    ln_b = np.asarray(inp['ln_b'], np.float32)

    d25 = np.zeros((128, 25 * 128), np.float32)
    for t in range(25):
        dy, dx = t // 5, t % 5
        d25[np.arange(128), 128 * t + np.arange(128)] = off_dw[:, 0, dy, dx]

    gsel = np.zeros((3, 3 * 128), np.float32)
    for g in range(3):
        gsel[g, 128 * g:128 * (g + 1)] = ln_g

    offx = np.zeros((128, 9), np.float32)
    offy = np.zeros((128, 9), np.float32)
    for g in range(3):
        offx[:, 3 * g + g] = off_w[0, :] * 15.5
        offy[:, 3 * g + g] = off_w[1, :] * 15.5

    return dict(
        wqt=np.ascontiguousarray(Wq.T), wkt=np.ascontiguousarray(Wk.T),
        wvt=np.ascontiguousarray(Wv.T), wot=np.ascontiguousarray(Wo.T),
        d25=d25, dwb=np.asarray(inp['off_dw_b'], np.float32).reshape(1, 128),
        gsel=gsel, lnb_neg=(-ln_b).reshape(1, 128), offx=offx, offy=offy,
        bq=np.asarray(inp['bq'], np.float32).reshape(1, C),
        bk=np.asarray(inp['bk'], np.float32).reshape(1, C),
        bv=np.asarray(inp['bv'], np.float32).reshape(1, C),
        bo=np.asarray(inp['bo'], np.float32).reshape(1, C))


def build_nc():
    nc = bacc.Bacc("TRN2", target_bir_lowering=False)
    g = {}

    def din(name, shape, dty=f32r):
        g[name] = nc.dram_tensor(name, list(shape), dty, kind="ExternalInput")

    din('x1c', (C, NS))
    din('x2c', (C, NS), dt.float32)
    for n in ('wqt', 'wkt', 'wvt', 'wot'):
        din(n, (C, C))
    for n in ('bq', 'bk', 'bv', 'bo'):
        din(n, (1, C))
    din('d25', (128, 25 * 128)); din('dwb', (1, 128))
    din('gsel', (3, 3 * 128)); din('lnb_neg', (1, 128))
    din('offx', (128, 9)); din('offy', (128, 9))
    din('grid2x', (3, NS), dt.float32); din('grid2y', (3, NS), dt.float32)
    din('ones_row', (1, NS)); din('ones_col12', (128, 2 * HEADS))
    din('zeros_pad', (1, 32 * 36))
    din('ident', (128, 128)); din('e3div', (128, 9)); din('sel3', (3, 3 * 128))
    yout = nc.dram_tensor("yout", [C, NS], dt.float32, kind="ExternalOutput")
    scr = nc.dram_tensor("scr_idx", [12, NS], dt.int16, kind="Internal")

    with tile.TileContext(nc) as tc:
        _emit(nc, tc, g, yout, scr)
    nc.finalize()
    return nc


def _emit(nc, tc, g, yout, scr):
    with ExitStack() as ctx:
        persist = ctx.enter_context(tc.tile_pool(name="persist", bufs=1))

        def load(pool, name, shape, dty=f32r, src=None):
            t = pool.tile(list(shape), dty, name=f"t_{name}")
            nc.sync.dma_start(out=t, in_=(src if src is not None else g[name])[:, :])
            return t

        nc.gpsimd.load_library(library_config.ap_gather)

        x1 = [load(persist, f'x1_{i}', (128, NS), f32r, g['x1c'][128 * i:128 * (i + 1), :]) for i in range(3)]
        wot = [load(persist, f'wot{i}', (128, C), f32r, g['wot'][128 * i:128 * (i + 1), :]) for i in range(3)]
        bo = load(persist, 'bo', (1, C))
        ones_row = load(persist, 'ones_row', (1, NS))
        ident = load(persist, 'ident', (128, 128))
        q_sb = [persist.tile([128, H, 36], f32r, name=f"q_sb{i}") for i in range(3)]
        for i in range(3):
            zsrc = bass.AP(tensor=g['zeros_pad'], offset=0, ap=[[0, 128], [1, 32 * 36]])
            nc.sync.dma_start(out=q_sb[i].rearrange("p y x -> p (y x)"), in_=zsrc)

        # pools with phase-scoped lifetimes
        midpool = ctx.enter_context(tc.tile_pool(name="midpool", bufs=1))
        samp = [midpool.tile([128, NS], f32r, name=f"samp{i}") for i in range(3)]
        wcs = [midpool.tile([3, NS], f32r, name=f"wc{i}") for i in range(4)]
        epsb3 = persist.tile([3, 1], dt.float32)
        nc.vector.memset(epsb3, EPS)

        # ================= q projection =================
        with tc.tile_pool(name="qproj_ps", bufs=2, space="PSUM") as qps, \
             tc.tile_pool(name="qw", bufs=1) as qw:
            wqt = [load(qw, f'wqt{i}', (128, C), f32r, g['wqt'][128 * i:128 * (i + 1), :]) for i in range(3)]
            bq = load(qw, 'bq', (1, C))
            for oc in range(3):
                p = qps.tile([128, NS], dt.float32, tag="qp", name=f"qp{oc}")
                for half in range(2):
                    sl = slice(512 * half, 512 * (half + 1))
                    for ic in range(3):
                        nc.tensor.matmul(p[:, sl], wqt[ic][:, 128 * oc:128 * (oc + 1)],
                                         x1[ic][:, sl], start=(ic == 0), stop=False)
                    nc.tensor.matmul(p[:, sl], bq[:, 128 * oc:128 * (oc + 1)],
                                     ones_row[:, sl], start=False, stop=True)
                nc.vector.tensor_copy(q_sb[oc][:, :, 2:34], p.rearrange("p (y x) -> p y x", y=H))

        # ============ offset net: conv + stats ============
        t_sb = []
        tpool = ctx.enter_context(tc.tile_pool(name="tpool", bufs=1))
        lnpool = ctx.enter_context(tc.tile_pool(name="lnpool", bufs=1))
        gelupool = ctx.enter_context(tc.tile_pool(name="gelupool", bufs=1))
        gelu_sb = [gelupool.tile([128, NS], f32r, name=f"gelu{i}") for i in range(3)]
        with tc.tile_pool(name="stats_ps", bufs=1, space="PSUM") as sps, \
             tc.tile_pool(name="convw", bufs=1) as convw:
            d25 = load(convw, 'd25', (128, 25 * 128))
            dwb = load(convw, 'dwb', (1, 128))
            e3div = load(convw, 'e3div', (128, 9))
            stats = sps.tile([3, 2 * NS], dt.float32)
            with tc.tile_pool(name="conv_ps", bufs=2, space="PSUM") as cps, \
                 tc.tile_pool(name="conv_sb", bufs=2) as csb:
                for grp in range(3):
                    tp = cps.tile([128, NS], dt.float32, tag="tps", name=f"tps{grp}")
                    tpv = tp.rearrange("p (y x) -> p y x", y=H)
                    qv = q_sb[grp]  # [128, 32, 36], x data at cols 2..34
                    for half in range(2):
                        nc.tensor.matmul(tpv[:, 16 * half:16 * (half + 1), :],
                                         d25[:, 128 * 12:128 * 13],
                                         qv[:, 16 * half:16 * (half + 1), 2:34],
                                         start=True, stop=False)
                    for t in range(25):
                        if t == 12:
                            continue
                        dy, dx = t // 5 - 2, t % 5 - 2
                        y0, y1 = max(0, -dy), H - max(0, dy)
                        lhs = d25[:, 128 * t:128 * (t + 1)]
                        for ya, yb in ((y0, min(16, y1)), (max(16, y0), y1)):
                            if ya >= yb:
                                continue
                            nc.tensor.matmul(
                                tpv[:, ya:yb, :], lhs,
                                qv[:, ya + dy:yb + dy, 2 + dx:34 + dx],
                                start=False, stop=False, skip_group_check=True)
                    for half in range(2):
                        nc.tensor.matmul(tp[:, 512 * half:512 * (half + 1)], dwb[:, :],
                                         ones_row[:, 512 * half:512 * (half + 1)],
                                         start=False, stop=(half == 1),
                                         skip_group_check=True)
                    ts_ = tpool.tile([128, NS], f32r, name=f"tsb{grp}")
                    t2_ = csb.tile([128, NS], f32r, tag="t2sb", name=f"t2sb{grp}")
                    nc.vector.tensor_copy(ts_, tp)
                    nc.scalar.activation(t2_, tp, Act.Square)
                    t_sb.append(ts_)
                    for half in range(2):
                        sl = slice(512 * half, 512 * (half + 1))
                        sl2 = slice(NS + 512 * half, NS + 512 * (half + 1))
                        nc.tensor.matmul(stats[:, sl], e3div[:, 3 * grp:3 * (grp + 1)],
                                         ts_[:, sl], start=(grp == 0), stop=(grp == 2),
                                         skip_group_check=True)
                        nc.tensor.matmul(stats[:, sl2], e3div[:, 3 * grp:3 * (grp + 1)],
                                         t2_[:, sl], start=(grp == 0), stop=(grp == 2),
                                         skip_group_check=True)

            # ---- LN row math (stats psum still held; conv pools closed) ----
            with tc.tile_pool(name="ln_rows", bufs=1) as lrp:
                stats_sb = lrp.tile([3, 2 * NS], dt.float32)
                nc.vector.tensor_copy(stats_sb, stats)
                musq = lrp.tile([3, NS], dt.float32)
                nc.vector.tensor_mul(musq, stats_sb[:, 0:NS], stats_sb[:, 0:NS])
                var = lrp.tile([3, NS], dt.float32)
                nc.vector.tensor_tensor(out=var, in0=stats_sb[:, NS:2 * NS], in1=musq,
                                        op=Alu.subtract)
                lnrows = lnpool.tile([3, 2 * NS], f32r, name="lnrows")
                lnv = lrp.tile([3, NS], dt.float32)
                nc.scalar.activation(lnv, var, Act.Ln, bias=epsb3[:, :])
                nc.scalar.activation(lnrows[:, NS:2 * NS], lnv, Act.Exp, scale=-0.5)
                nc.vector.tensor_mul(lnrows[:, 0:NS], stats_sb[:, 0:NS],
                                     lnrows[:, NS:2 * NS])

        # ============ LN apply + gelu ============
        with tc.tile_pool(name="lnab_ps", bufs=2, space="PSUM") as lps, \
             tc.tile_pool(name="lnu_sb", bufs=2) as lsb:
            gsel = load(lnpool, 'gsel', (3, 3 * 128))
            lnb_neg = load(lnpool, 'lnb_neg', (1, 128))
            for grp in range(3):
                ap_ = lps.tile([128, NS], dt.float32, tag="lnA", name=f"lnA{grp}")
                bp_ = lps.tile([128, NS], dt.float32, tag="lnB", name=f"lnB{grp}")
                for half in range(2):
                    sl = slice(512 * half, 512 * (half + 1))
                    nc.tensor.matmul(ap_[:, sl], gsel[:, 128 * grp:128 * (grp + 1)],
                                     lnrows[:, NS + 512 * half:NS + 512 * (half + 1)],
                                     start=True, stop=True)
                    nc.tensor.matmul(bp_[:, sl], gsel[:, 128 * grp:128 * (grp + 1)],
                                     lnrows[:, sl], start=True, stop=False)
                    nc.tensor.matmul(bp_[:, sl], lnb_neg[:, :], ones_row[:, sl],
                                     start=False, stop=True)
                u = lsb.tile([128, NS], dt.float32, tag="u", name=f"u{grp}")
                nc.vector.tensor_mul(u, t_sb[grp], ap_)
                gin = lsb.tile([128, NS], dt.float32, tag="gin", name=f"gin{grp}")
                nc.vector.tensor_tensor(out=gin, in0=u, in1=bp_, op=Alu.subtract)
                nc.scalar.activation(gelu_sb[grp], gin, Act.Gelu_apprx_tanh)

        # ============ offsets -> positions -> idx + weights ============
        with tc.tile_pool(name="off_ps", bufs=1, space="PSUM") as ops_, \
             tc.tile_pool(name="pos", bufs=1) as pos:
            offx = load(pos, 'offx', (128, 9)); offy = load(pos, 'offy', (128, 9))
            grid2x = load(pos, 'grid2x', (3, NS), dt.float32)
            grid2y = load(pos, 'grid2y', (3, NS), dt.float32)
            iw = {}
            for fam, selw, gridt in (('x', offx, grid2x), ('y', offy, grid2y)):
                p = ops_.tile([3, NS], dt.float32, name=f"offps_{fam}")
                for grp in range(3):
                    for half in range(2):
                        sl = slice(512 * half, 512 * (half + 1))
                        nc.tensor.matmul(p[:, sl], selw[:, 3 * grp:3 * (grp + 1)],
                                         gelu_sb[grp][:, sl], start=(grp == 0),
                                         stop=(grp == 2), skip_group_check=True)
                i_ = pos.tile([3, NS], dt.float32, tag="i_", name=f"i_{fam}")
                nc.vector.tensor_tensor(out=i_, in0=gridt, in1=p, op=Alu.add)
                nc.vector.tensor_scalar(i_, i_, 0.0, 31.0, Alu.max, Alu.min)
                i0 = pos.tile([3, NS], dt.int32, name=f"i0_{fam}")
                nc.vector.tensor_scalar(i0, i_, -0.49999985, None, Alu.add)
                i0f = pos.tile([3, NS], dt.float32, tag="i0f", name=f"i0f_{fam}")
                nc.vector.tensor_copy(i0f, i0)
                w_ = pos.tile([3, NS], dt.float32, name=f"w_{fam}")
                nc.vector.tensor_tensor(out=w_, in0=i_, in1=i0f, op=Alu.subtract)
                cw = pos.tile([3, NS], dt.float32, name=f"cw_{fam}")
                nc.vector.tensor_scalar(cw, w_, -1.0, 1.0, Alu.mult, Alu.add)
                i1 = pos.tile([3, NS], dt.int32, name=f"i1_{fam}")
                nc.vector.tensor_scalar(i1, i0, 1, 31, Alu.add, Alu.min)
                iw[fam] = {'i': (i0, i1), 'w': (cw, w_)}

            for ci, (yi, xi) in enumerate(((0, 0), (0, 1), (1, 0), (1, 1))):
                fidx = pos.tile([3, NS], dt.int16, tag="fidx", name=f"fidx{ci}")
                nc.vector.scalar_tensor_tensor(fidx, iw['y']['i'][yi], 32.0,
                                               iw['x']['i'][xi], Alu.mult, Alu.add)
                nc.sync.dma_start(out=scr[3 * ci:3 * (ci + 1), :], in_=fidx)
                nc.vector.tensor_mul(wcs[ci], iw['y']['w'][yi], iw['x']['w'][xi])

        # ============ idx wrap + gathers + bilinear combine ============
        with tc.tile_pool(name="idxp", bufs=1) as idxp, \
             tc.tile_pool(name="x2p", bufs=2) as x2p, \
             tc.tile_pool(name="vg", bufs=2) as vg, \
             tc.tile_pool(name="wb_ps", bufs=2, space="PSUM") as wps:
            sel3 = load(idxp, 'sel3', (3, 3 * 128))
            idx_sb = idxp.tile([128, 12, 64], dt.int16)
            for b in range(8):
                for cg in range(12):
                    in_r = bass.AP(tensor=scr, offset=cg * NS, ap=[[1, 16], [16, 64]])
                    nc.gpsimd.dma_start(out=idx_sb[16 * b:16 * (b + 1), cg, :], in_=in_r)

            for grp in range(3):
                x2t = x2p.tile([128, NS], dt.float32, tag="x2t", name=f"x2t{grp}")
                nc.sync.dma_start(out=x2t, in_=g['x2c'][128 * grp:128 * (grp + 1), :])
                v4 = vg.tile([128, 4, NS], dt.float32, tag="v4", name=f"v4_{grp}")
                for ci in range(4):
                    nc.gpsimd.ap_gather(v4[:, ci, :], x2t, idx_sb[:, 3 * ci + grp, :],
                                        channels=128, num_elems=NS, d=1, num_idxs=NS)
                prs = []
                for pair in range(2):
                    wbp = wps.tile([128, 2 * NS], dt.float32, tag="wb",
                                   name=f"wb{grp}_{pair}")
                    for ci2 in range(2):
                        ci = 2 * pair + ci2
                        for half in range(2):
                            nc.tensor.matmul(
                                wbp[:, NS * ci2 + 512 * half:NS * ci2 + 512 * (half + 1)],
                                sel3[:, 128 * grp:128 * (grp + 1)],
                                wcs[ci][:, 512 * half:512 * (half + 1)],
                                start=True, stop=True, skip_group_check=True)
                    pr = vg.tile([128, 2 * NS], dt.float32, tag="pr",
                                 name=f"pr{grp}_{pair}")
                    nc.vector.tensor_mul(
                        pr, v4.rearrange("p c n -> p (c n)")[:, 2 * NS * pair:2 * NS * (pair + 1)],
                        wbp)
                    prs.append(pr)
                acc1 = vg.tile([128, NS], dt.float32, tag="acc1", name=f"acc1_{grp}")
                acc2 = vg.tile([128, NS], dt.float32, tag="acc2", name=f"acc2_{grp}")
                nc.vector.tensor_add(acc1, prs[0][:, 0:NS], prs[0][:, NS:2 * NS])
                nc.vector.tensor_add(acc2, prs[1][:, 0:NS], prs[1][:, NS:2 * NS])
                nc.vector.tensor_add(samp[grp], acc1, acc2)

        # ============ k and vT projections ============
        late = ctx.enter_context(tc.tile_pool(name="late", bufs=1))
        k_sb = [late.tile([128, NS], f32r, name=f"k_sb{i}") for i in range(3)]
        vta = [late.tile([128, HEADS, 34], f32r, name=f"vta{i}") for i in range(KC)]
        attn = [late.tile([128, NS], f32r, name=f"attn{i}") for i in range(3)]
        with tc.tile_pool(name="kv_ps", bufs=2, space="PSUM") as kps, \
             tc.tile_pool(name="kvw", bufs=1) as kvw:
            wkt = [load(kvw, f'wkt{i}', (128, C), f32r, g['wkt'][128 * i:128 * (i + 1), :]) for i in range(3)]
            wvt = [load(kvw, f'wvt{i}', (128, C), f32r, g['wvt'][128 * i:128 * (i + 1), :]) for i in range(3)]
            bk = load(kvw, 'bk', (1, C)); bv = load(kvw, 'bv', (1, C))
            ones_col12 = load(kvw, 'ones_col12', (128, 2 * HEADS))
            for oc in range(3):
                p = kps.tile([128, NS], dt.float32, tag="kp", name=f"kp{oc}")
                for half in range(2):
                    sl = slice(512 * half, 512 * (half + 1))
                    for ic in range(3):
                        nc.tensor.matmul(p[:, sl], wkt[ic][:, 128 * oc:128 * (oc + 1)],
                                         samp[ic][:, sl], start=(ic == 0), stop=False)
                    nc.tensor.matmul(p[:, sl], bk[:, 128 * oc:128 * (oc + 1)],
                                     ones_row[:, sl], start=False, stop=True)
                nc.vector.tensor_copy(k_sb[oc], p)
            for nt in range(KC):
                p = kps.tile([128, C], dt.float32, tag="vp", name=f"vp{nt}")
                for ic in range(3):
                    nc.tensor.matmul(p, samp[ic][:, 128 * nt:128 * (nt + 1)],
                                     wvt[ic], start=(ic == 0), stop=False)
                nc.tensor.matmul(p, ones_row[:, 0:128], bv, start=False, stop=True)
                nc.sync.dma_start(out=vta[nt][:, :, 32:34], in_=ones_col12[:, :])
                nc.vector.tensor_copy(vta[nt][:, :, 0:32],
                                      p.rearrange("p (h d) -> p h d", h=HEADS))

        # ============ attention ============
        with tc.tile_pool(name="sc_ps", bufs=1, space="PSUM") as scps, \
             tc.tile_pool(name="at_ps", bufs=2, space="PSUM") as atps, \
             tc.tile_pool(name="tr_ps", bufs=2, space="PSUM") as trps, \
             tc.tile_pool(name="pt", bufs=2) as ptp, \
             tc.tile_pool(name="an", bufs=2) as anp:
            for qq in range(QQ):
                for quad in range(3):
                    pts = ptp.tile([128, 4, KC, 256], f32r, tag="pts",
                                   name=f"pts{qq}_{quad}")
                    for kcp in range(4):
                        wv = scps.tile([128, 4, 2, 256], dt.float32, tag="wv",
                                       name=f"wv{qq}_{quad}_{kcp}")
                        for kc2 in range(2):
                            kc = 2 * kcp + kc2
                            for h4 in range(4):
                                nc.tensor.matmul(
                                    wv[:, h4, kc2, :],
                                    k_sb[quad][32 * h4:32 * (h4 + 1), 128 * kc:128 * (kc + 1)],
                                    q_sb[quad][32 * h4:32 * (h4 + 1), 8 * qq:8 * (qq + 1), 2:34],
                                    start=True, stop=True, tile_position=(32 * h4, 0),
                                    skip_group_check=True)
                        nc.scalar.activation(pts[:, :, 2 * kcp:2 * (kcp + 1), :], wv,
                                             Act.Exp, scale=SCALE)
                    at = atps.tile([128, 2, 136], dt.float32, tag="at",
                                   name=f"at{qq}_{quad}")
                    for t in range(2):
                        for h4 in range(4):
                            for kc in range(KC):
                                nc.tensor.matmul(
                                    at[:, t, 34 * h4:34 * (h4 + 1)],
                                    pts[:, h4, kc, 128 * t:128 * (t + 1)],
                                    vta[kc][:, 4 * quad + h4, :],
                                    start=(kc == 0), stop=(kc == KC - 1),
                                    skip_group_check=True)
                    rc = anp.tile([128, 2, 4, 1], dt.float32, tag="rc",
                                  name=f"rc{qq}_{quad}")
                    atv = at.rearrange("p t (h e) -> p t h e", h=4)
                    nc.vector.reciprocal(out=rc.rearrange("p t h o -> p t (h o)"),
                                         in_=atv[:, :, :, 32])
                    ansb = anp.tile([128, 2, 128], f32r, tag="ansb",
                                    name=f"an{qq}_{quad}")
                    nc.vector.tensor_mul(
                        ansb.rearrange("p t (h d) -> p t h d", h=4),
                        atv[:, :, :, 0:32],
                        rc.to_broadcast((128, 2, 4, 32)))
                    for t in range(2):
                        trp = trps.tile([128, 128], f32r, tag="trp",
                                        name=f"tr{qq}_{quad}_{t}")
                        nc.tensor.transpose(trp, ansb[:, t, :], ident)
                        dst = attn[quad][:, 256 * qq + 128 * t:256 * qq + 128 * (t + 1)]
                        if (t + quad) % 2 == 0:
                            nc.vector.tensor_copy(dst, trp)
                        else:
                            nc.scalar.copy(dst, trp)

        # ============ output projection + residual ============
        with tc.tile_pool(name="o_ps", bufs=2, space="PSUM") as ops2, \
             tc.tile_pool(name="o_sb", bufs=2) as osb:
            for oc in range(3):
                p = ops2.tile([128, NS], dt.float32, tag="op", name=f"op{oc}")
                for half in range(2):
                    sl = slice(512 * half, 512 * (half + 1))
                    for ic in range(3):
                        nc.tensor.matmul(p[:, sl], wot[ic][:, 128 * oc:128 * (oc + 1)],
                                         attn[ic][:, sl], start=(ic == 0), stop=False)
                    nc.tensor.matmul(p[:, sl], bo[:, 128 * oc:128 * (oc + 1)],
                                     ones_row[:, sl], start=False, stop=True)
                o = osb.tile([128, NS], dt.float32, tag="ot", name=f"ot{oc}")
                nc.vector.tensor_tensor(out=o, in0=x1[oc], in1=p, op=Alu.add)
                nc.sync.dma_start(out=yout[128 * oc:128 * (oc + 1), :], in_=o)


def kernel(**inputs):
    x1 = np.asarray(inputs['x1'], np.float32)
    x2 = np.asarray(inputs['x2'], np.float32)
    wd = _host_weights(inputs)
    grid2x, grid2y, ones_row, ones_col12, ident, e3div, sel3, zeros_pad = _host_consts()

    if 'nc' not in _CACHED:
        _CACHED['nc'] = build_nc()
    nc = _CACHED['nc']

    shared = dict(wqt=wd['wqt'], wkt=wd['wkt'], wvt=wd['wvt'], wot=wd['wot'],
                  bq=wd['bq'], bk=wd['bk'], bv=wd['bv'], bo=wd['bo'],
                  d25=wd['d25'], dwb=wd['dwb'], gsel=wd['gsel'],
                  lnb_neg=wd['lnb_neg'], offx=wd['offx'], offy=wd['offy'],
                  grid2x=grid2x, grid2y=grid2y, ones_row=ones_row,
                  ones_col12=ones_col12, ident=ident, e3div=e3div, sel3=sel3,
                  zeros_pad=zeros_pad)
    in_maps = []
    for b in range(NCORES):
        m = dict(shared)
        m['x1c'] = np.ascontiguousarray(x1[b].reshape(C, NS))
        m['x2c'] = np.ascontiguousarray(x2[b].reshape(C, NS))
        in_maps.append(m)

    res = bass_utils.run_bass_kernel_spmd(nc, in_maps, core_ids=list(range(NCORES)))
    out = np.stack([res.results[b]['yout'].reshape(C, H, W) for b in range(NCORES)])
    return out.astype(np.float32)
